# revision 1
# baseline (speedup 1.0000x reference)
"""nn_LRSA3D on 8 NeuronCores.

Stage A: windowed attention, window-sharded (216 windows -> 27/core).
Stage B: overlap-add + ConvFFN, slab-sharded (b x 4 h-slabs of 9 rows,
         halo 2 for the 5^3 depthwise conv). Host does only gather /
         slice / concat; all math runs on device via pmap.
"""
import numpy as np
import jax
import jax.numpy as jnp
from jax import lax
from functools import partial

LN_EPS = 1e-5
HEADS = 2
B, C, T, H, W = 2, 64, 16, 36, 36
PS, STEP = 8, 6
ST_T = np.array([0, 6, 8], np.int32)
ST_HW = np.array([0, 6, 12, 18, 24, 28], np.int32)
NT, NH, NW = 3, 6, 6
NWIN = NT * NH * NW            # 108 per batch
NCORES = 8
WPC = B * NWIN // NCORES       # 27 windows per core
SLAB = 9                       # h rows per stage-B core
HALO = 2
HS = SLAB + 2 * HALO           # 13 rows incl. halo
# h-window sets per slab (windows intersecting [h0-2, h0+11)), padded to 4
HSETS = [(0, 1, 0, 0), (0, 1, 2, 3), (2, 3, 4, 5), (3, 4, 5, 5)]
HSET_VALID = [(1, 1, 0, 0), (1, 1, 1, 1), (1, 1, 1, 1), (1, 1, 1, 0)]


def _divisor(L, step, ps):
    d = np.ones(L, np.float32)
    for k in range(step, L + step - ps, step):
        top = L - ps if k + ps > L else k
        d[top:top + ps - step] *= 2.0
    return d


def _onehot(starts, L):
    n = len(starts)
    g = np.zeros((n, PS, L), np.float32)
    for i, s in enumerate(starts):
        for p in range(PS):
            g[i, p, s + p] = 1.0
    return g


def _ln(x, g, b):
    m = jnp.mean(x, axis=-1, keepdims=True)
    v = jnp.mean((x - m) ** 2, axis=-1, keepdims=True)
    return (x - m) / jnp.sqrt(v + LN_EPS) * g + b


@partial(jax.pmap, in_axes=(0, None, None, None, None, None, None, None))
def _stage_a(tok, ln1_g, ln1_b, wq, wk, wv, wproj, scale):
    # tok: [WPC, 512, 64]
    y = _ln(tok, ln1_g, ln1_b)
    q = (y @ wq).reshape(WPC, PS**3, HEADS, -1).transpose(0, 2, 1, 3)
    k = (y @ wk).reshape(WPC, PS**3, HEADS, -1).transpose(0, 2, 1, 3)
    v = (y @ wv).reshape(WPC, PS**3, HEADS, -1).transpose(0, 2, 1, 3)
    s = jnp.einsum('bhnd,bhmd->bhnm', q, k) * scale
    p = jax.nn.softmax(s, axis=-1)
    o = jnp.einsum('bhnm,bhmd->bhnd', p, v)
    o = o.transpose(0, 2, 1, 3).reshape(WPC, PS**3, C) @ wproj
    return o + tok


@partial(jax.pmap,
         in_axes=(0, 0, 0, None, None, None, None, None, None, None, None,
                  None, None))
def _stage_b(wins, g_h, rdiv, g_t, g_w, ln2_g, ln2_b, fc1_w, fc1_b, dw_w,
             dw_b, fc2_w, fc2_b):
    # wins: [3, 4, 6, 512, 64]  (wt, wh-subset, ww windows for this slab)
    # g_h: [4, 8, HS]  one-hot (zero rows for padded/invalid entries)
    # rdiv: [T, HS, W]  reciprocal divisor (1 outside valid rows)
    w6 = wins.reshape(NT, 4, NW, PS, PS, PS, C)
    acc = jnp.einsum('abcpqre,apT,bqU,crV->eTUV', w6, g_t, g_h, g_w)
    xr = acc * rdiv[None]                      # [C, T, HS, W]
    xtok = xr.reshape(C, T * HS * W).T         # [V, C]
    y = _ln(xtok, ln2_g, ln2_b)
    y = jax.nn.gelu(y @ fc1_w + fc1_b, approximate=False)
    hid = y.shape[-1]
    yv = y.T.reshape(1, hid, T, HS, W)
    dw = lax.conv_general_dilated(
        yv, dw_w, (1, 1, 1), [(2, 2), (2, 2), (2, 2)],
        feature_group_count=hid,
        dimension_numbers=('NCDHW', 'OIDHW', 'NCDHW'))
    dw = jax.nn.gelu(dw + dw_b[None, :, None, None, None], approximate=False)
    y = y + dw.reshape(hid, T * HS * W).T
    y = y @ fc2_w + fc2_b
    out = (y + xtok).T.reshape(C, T, HS, W)
    return out[:, :, HALO:HALO + SLAB, :]      # [C, T, 9, W]


def kernel(x, ln1_g, ln1_b, wq, wk, wv, wproj, ln2_g, ln2_b,
           fc1_w, fc1_b, dw_w, dw_b, fc2_w, fc2_b, ps):
    x = np.asarray(x, np.float32)
    assert int(ps) == PS and x.shape == (B, C, T, H, W)

    # ---- host: gather windows (pure indexing) ----
    off = np.arange(PS)
    it = ST_T[:, None] + off
    ihw = ST_HW[:, None] + off
    xt = x[:, :, it]                        # b,c,nt,ps,h,w
    xth = xt[:, :, :, :, ihw]               # b,c,nt,ps,nh,ps,w
    xthw = xth[:, :, :, :, :, :, ihw]       # b,c,nt,ps,nh,ps,nw,ps
    tokens = xthw.transpose(0, 2, 4, 6, 3, 5, 7, 1).reshape(
        B * NWIN, PS**3, C)                 # [216, 512, 64]

    scale = np.float32(1.0 / np.sqrt(wq.shape[1] // HEADS))
    tok_sh = tokens.reshape(NCORES, WPC, PS**3, C)
    outs = _stage_a(tok_sh, ln1_g, ln1_b, wq, wk, wv, wproj, scale)
    wout = np.asarray(outs).reshape(B, NT, NH, NW, PS**3, C)

    # ---- host: reshard windows -> (b, h-slab) cores ----
    wins, ghs, rdivs = [], [], []
    dt = _divisor(T, STEP, PS)
    dh = _divisor(H, STEP, PS)
    dw_ = _divisor(W, STEP, PS)
    div = dt[:, None, None] * dh[None, :, None] * dw_[None, None, :]
    g_t = _onehot(ST_T, T)
    g_w = _onehot(ST_HW, W)
    for b in range(B):
        for s in range(4):
            h0 = s * SLAB
            hset = HSETS[s]
            wins.append(wout[b][:, list(hset)])       # [3,4,6,512,64]
            gh = np.zeros((4, PS, HS), np.float32)
            rd = np.ones((T, HS, W), np.float32)
            for j, wh in enumerate(hset):
                if not HSET_VALID[s][j]:
                    continue
                st = ST_HW[wh]
                for p in range(PS):
                    r = st + p - (h0 - HALO)
                    if 0 <= r < HS:
                        gh[j, p, r] = 1.0
            for r in range(HS):
                hr = h0 - HALO + r
                if 0 <= hr < H:
                    rd[:, r, :] = 1.0 / div[:, hr, :]
            ghs.append(gh)
            rdivs.append(rd)

    wins = np.stack(wins)
    ghs = np.stack(ghs)
    rdivs = np.stack(rdivs)
    slabs = _stage_b(wins, ghs, rdivs, g_t, g_w, ln2_g, ln2_b,
                     fc1_w, fc1_b, np.asarray(dw_w, np.float32), dw_b,
                     fc2_w, fc2_b)
    slabs = np.asarray(slabs).reshape(B, 4, C, T, SLAB, W)
    out = np.concatenate([slabs[:, i] for i in range(4)], axis=3)
    return np.ascontiguousarray(out.astype(np.float32))
